# revision 1
# baseline (speedup 1.0000x reference)
"""Sparse-attention head kernel for Trainium2, data-parallel over batch on 8 cores.

Math per batch b (see reference):
  q,k,v = x @ W{q,k,v}.T + b{q,k,v}          # [T, 64]
  qg    = q[keep]                            # [K=T/2, 64]
  att   = softmax(mask(qg @ k.T / sqrt(C)))  # [K, T], row i allows t <= keep[i]
  out   = att @ v                            # [K, 64]

Device strategy (per core, one batch):
  - host uploads x[b].T in bf16 (contraction dim C on SBUF partitions)
  - projections as qkv_nat[t,192] = sum_c xT_chunk.T @ Wchunk (+ ones x bias)
  - k transposed on PE; q round-trips DRAM for an indirect row gather by keep
  - transposed attention: S_T[t,q] = kT.T @ qgT, E = exp(S/sqrt(C)) * mask,
    out_T[65,q] = sum_t [v|1].T @ E  (row 64 = softmax denominator)
  - attention for a q-chunk is emitted as soon as its t-prefix is projected,
    so it overlaps the tail of the x load
  - PE-transpose out_T, divide by denominator, DMA out
All matmul inputs bf16 (fp32 accumulation in PSUM); final epilogue in fp32.
"""

import math
import os

if "JAX_PLATFORMS" not in os.environ:
    os.environ["JAX_PLATFORMS"] = "axon,cpu"

import numpy as np
import ml_dtypes

B, T, C = 8, 4096, 1024
HS = 64
KQ = T // 2  # 2048 gathered query rows
NCORES = 8
SCALE = float(C) ** -0.5
QC = 512   # attention q-chunk (matmul moving width)
BF16 = ml_dtypes.bfloat16
NQC = KQ // QC  # 4


def _keep_indices(t):
    a = math.ceil(t / 4)
    keep = [t - 1 - x for x in range(a)]
    keep += [t - 1 - math.ceil(3 / a * (x - a) ** 2 + a) for x in range(a, math.ceil(t / 2))]
    return np.array(list(reversed(keep)), dtype=np.int64)


KEEP = _keep_indices(T)  # [KQ], ascending

# Static block classification at [t=128] x [q=128] granularity.
# block (tb, j): t in [128*tb, 128*tb+128), q rows j*128..j*128+127;
# allow iff t <= keep[q].
_NT = T // 128   # 32
_NJ = KQ // 128  # 16
_FULL, _BOUND, _DEAD = 0, 1, 2
_BLOCK_KIND = np.empty((_NT, _NJ), dtype=np.int64)
_MASK_IDX = {}
for _tb in range(_NT):
    for _j in range(_NJ):
        qlo = KEEP[_j * 128]
        qhi = KEEP[_j * 128 + 127]
        if 128 * _tb + 127 <= qlo:
            _BLOCK_KIND[_tb, _j] = _FULL
        elif 128 * _tb > qhi:
            _BLOCK_KIND[_tb, _j] = _DEAD
        else:
            _BLOCK_KIND[_tb, _j] = _BOUND
            _MASK_IDX[(_tb, _j)] = len(_MASK_IDX)
_NMASK = len(_MASK_IDX)

# t-blocks needed per q-chunk, and first alive j-subblock per (qc, tb)
_NTB_QC = [int(KEEP[qc * QC + QC - 1]) // 128 + 1 for qc in range(NQC)]


def _alive_j0(qc, tb):
    # sub-blocks j in [4qc, 4qc+4); dead ones form a prefix (keep ascending)
    for jj in range(QC // 128):
        if _BLOCK_KIND[tb, qc * (QC // 128) + jj] != _DEAD:
            return jj
    return QC // 128


def _host_masks():
    m = np.zeros((128, _NMASK * 128), dtype=np.float32)
    for (tb, j), idx in _MASK_IDX.items():
        tvals = 128 * tb + np.arange(128)[:, None]
        kvals = KEEP[j * 128:(j + 1) * 128][None, :]
        m[:, idx * 128:(idx + 1) * 128] = (tvals <= kvals).astype(np.float32)
    return m.astype(BF16)


_prog_cache = {}
TRACE = False          # set by test harness to collect an NTFF profile
TRACE_KW = {}
LAST_RESULTS = None    # BassKernelResults of the most recent kernel() call


def _build_program(reps=1):
    import concourse.bass as bass
    import concourse.mybir as mybir
    import concourse.tile as tile
    from concourse import bacc
    from concourse.masks import make_identity

    dt = mybir.dt
    f32, bf16, u32 = dt.float32, dt.bfloat16, dt.uint32
    Alu = mybir.AluOpType
    Act = mybir.ActivationFunctionType

    nc = bacc.Bacc("TRN2", target_bir_lowering=False, debug=False,
                   enable_partition_id=False)

    xt_d = nc.dram_tensor("xt", [C, T], bf16, kind="ExternalInput").ap()
    wpack_d = nc.dram_tensor("wpack", [128, 8 * 192], f32, kind="ExternalInput").ap()
    bias_d = nc.dram_tensor("bias", [1, 192], f32, kind="ExternalInput").ap()
    masks_d = nc.dram_tensor("masks", [128, _NMASK * 128], bf16, kind="ExternalInput").ap()
    keep_d = nc.dram_tensor("keepidx", [128, _NJ], u32, kind="ExternalInput").ap()
    out_d = nc.dram_tensor("out", [KQ, HS], f32, kind="ExternalOutput").ap()

    NTC = 4        # xt DMA t-chunks
    TCW = T // NTC  # 1024

    with tile.TileContext(nc) as tc:
        with (
            tc.tile_pool(name="const", bufs=1) as constp,
            tc.tile_pool(name="xt", bufs=1) as xtp,
            tc.tile_pool(name="proj", bufs=1) as projp,
            tc.tile_pool(name="dram", bufs=1, space="DRAM") as dramp,
            tc.tile_pool(name="psA", bufs=2, space="PSUM") as psA,
            tc.tile_pool(name="psB", bufs=1, space="PSUM") as psB,
            tc.tile_pool(name="psS", bufs=2, space="PSUM") as psS,
            tc.tile_pool(name="psO", bufs=1, space="PSUM") as psO,
            tc.tile_pool(name="work", bufs=2) as workp,
            tc.tile_pool(name="ework", bufs=4) as ep,
        ):
            # ---- constants (SWDGE: keep the HWDGE queues free for xt bulk) ----
            ident_b = constp.tile([128, 128], bf16)
            make_identity(nc, ident_b)
            ident_f = constp.tile([128, 128], f32)
            make_identity(nc, ident_f)

            wpack_sb = constp.tile([128, 8 * 192], bf16)
            nc.gpsimd.dma_start(out=wpack_sb, in_=wpack_d)
            w_sb = [wpack_sb[:, c * 192:(c + 1) * 192] for c in range(8)]
            bias_bc = constp.tile([128, 192], bf16)
            nc.gpsimd.dma_start(out=bias_bc, in_=bias_d.to_broadcast([128, 192]))

            mask_big = constp.tile([128, _NMASK * 128], bf16)
            nc.gpsimd.dma_start(out=mask_big, in_=masks_d)
            keep_big = constp.tile([128, _NJ], u32)
            nc.gpsimd.dma_start(out=keep_big, in_=keep_d)

            # ---- per-repetition kernel body (reps>1 only for timing) ----
            def emit_once():
                # persistent tensors: same pool tags each rep -> slots reused,
                # reps serialize on the data naturally
                xt_big = xtp.tile([128, 8 * T], bf16, name="xt_big", tag="xt_big")
                kt_sb = projp.tile([64, T], bf16, name="kt_sb", tag="kt_sb")
                qgt_sb = projp.tile([64, KQ], bf16, name="qgt_sb", tag="qgt_sb")
                vext_sb = [projp.tile([128, HS + 1], bf16, name=f"vext_{tb}",
                                      tag=f"vext_{tb}") for tb in range(_NT)]
                qscr = dramp.tile([T, HS], bf16, name="qscr", tag="qscr")

                def xt_sl(c, lo, hi):
                    return xt_big[:, c * T + lo: c * T + hi]

                wave_state = {}

                def emit_gather(qc):
                    ntb = _NTB_QC[qc]
                    qsrc = qscr[0:ntb * 128, :]  # dep only on projected prefix
                    for jj in range(QC // 128):
                        j = qc * (QC // 128) + jj
                        qg_g = workp.tile([128, HS], bf16, name="qg_g", tag="qg")
                        nc.gpsimd.indirect_dma_start(
                            out=qg_g, out_offset=None, in_=qsrc,
                            in_offset=bass.IndirectOffsetOnAxis(
                                ap=keep_big[:, j:j + 1], axis=0),
                        )
                        ps_qgt = psB.tile([64, 128], bf16, name="ps_qgt", tag="small")
                        nc.tensor.transpose(ps_qgt, qg_g, ident_b)
                        nc.vector.tensor_copy(qgt_sb[:, j * 128:(j + 1) * 128], ps_qgt)
                    wave_state[qc] = {"ps_o": None, "pv_pending": None}

                def emit_pair(qc, tba, tbb):
                    """ST pair + one exp + masks; emits previous pair's PVs."""
                    st = wave_state[qc]
                    if st["ps_o"] is None:
                        st["ps_o"] = psO.tile([HS + 1, QC], f32, name=f"ps_o_{qc}",
                                              tag="ps_o")
                    q0 = qc * QC
                    tbs = [tba] if tbb is None else [tba, tbb]
                    a0s = [_alive_j0(qc, tb) * 128 for tb in tbs]
                    width = QC * len(tbs)
                    ps_s = psS.tile([128, 2 * QC], f32, name="ps_s")
                    for i, tb in enumerate(tbs):
                        nc.tensor.matmul(
                            ps_s[:, i * QC + a0s[i]:(i + 1) * QC],
                            lhsT=kt_sb[:, tb * 128:(tb + 1) * 128],
                            rhs=qgt_sb[:, q0 + a0s[i]:q0 + QC], start=True, stop=True,
                        )
                    prev_pv = st["pv_pending"]
                    st["pv_pending"] = None
                    e_sb = ep.tile([128, 2 * QC], bf16, name="e_sb")
                    amin = min(a0s)
                    nc.scalar.activation(e_sb[:, amin:width], ps_s[:, amin:width],
                                         Act.Exp, scale=SCALE)
                    for i, tb in enumerate(tbs):
                        for jj in range(a0s[i] // 128, QC // 128):
                            j = q0 // 128 + jj
                            if _BLOCK_KIND[tb, j] == _BOUND:
                                midx = _MASK_IDX[(tb, j)]
                                o = i * QC + jj * 128
                                nc.vector.tensor_tensor(
                                    out=e_sb[:, o:o + 128], in0=e_sb[:, o:o + 128],
                                    in1=mask_big[:, midx * 128:(midx + 1) * 128],
                                    op=Alu.mult,
                                )
                    if prev_pv is not None:
                        emit_pv(qc, *prev_pv)
                    st["pv_pending"] = (tbs, e_sb, a0s)

                def emit_pv(qc, tbs, e_sb, a0s):
                    st = wave_state[qc]
                    ntb = _NTB_QC[qc]
                    for i, tb in enumerate(tbs):
                        nc.tensor.matmul(
                            st["ps_o"][:, a0s[i]:QC], lhsT=vext_sb[tb],
                            rhs=e_sb[:, i * QC + a0s[i]:(i + 1) * QC],
                            start=(tb == 0), stop=(tb == ntb - 1),
                        )

                def emit_epilogue(qc):
                    st = wave_state[qc]
                    if st["pv_pending"] is not None:
                        emit_pv(qc, *st["pv_pending"])
                        st["pv_pending"] = None
                    q0 = qc * QC
                    ps_o = st["ps_o"]
                    ot_sb = workp.tile([HS + 1, QC], f32, name="ot_sb", tag="ot")
                    nc.vector.tensor_copy(ot_sb, ps_o)
                    out4 = workp.tile([128, (QC // 128) * HS], f32,
                                      name="out4", tag="out4")
                    for jj in range(QC // 128):
                        ps_on = psB.tile([128, HS + 1], f32, name="ps_on", tag="small")
                        nc.tensor.transpose(
                            ps_on, ot_sb[:, jj * 128:(jj + 1) * 128],
                            ident_f[0:HS + 1, 0:HS + 1],
                        )
                        rec = workp.tile([128, 1], f32, name="rec", tag="rec")
                        nc.vector.reciprocal(rec, ps_on[:, HS:HS + 1])
                        nc.vector.tensor_scalar(
                            out=out4[:, jj * HS:(jj + 1) * HS], in0=ps_on[:, 0:HS],
                            scalar1=rec[:, :1], scalar2=None, op0=Alu.mult,
                        )
                    out_view = out_d[q0:q0 + QC, :].rearrange("(j p) d -> p j d", p=128)
                    nc.sync.dma_start(out=out_view,
                                      in_=out4.rearrange("p (j d) -> p j d",
                                                         j=QC // 128))

                pair_queue = []

                def emit_pairs(n):
                    for _ in range(min(n, len(pair_queue))):
                        item = pair_queue.pop(0)
                        if item[0] == "pair":
                            emit_pair(*item[1:])
                        else:
                            emit_epilogue(item[1])

                def queue_wave(qc):
                    ntb = _NTB_QC[qc]
                    for tb in range(0, ntb - 1, 2):
                        pair_queue.append(("pair", qc, tb, tb + 1))
                    if ntb % 2:
                        pair_queue.append(("pair", qc, ntb - 1, None))
                    pair_queue.append(("epi", qc))

                # ---- load xT + projections, attention interleaved ----
                qk4 = None
                for tci in range(NTC):
                    lo, hi = tci * TCW, (tci + 1) * TCW
                    for c in range(8):
                        nc.sync.dma_start(out=xt_sl(c, lo, hi),
                                          in_=xt_d[c * 128:(c + 1) * 128, lo:hi])
                    for tb in range(tci * (TCW // 128), (tci + 1) * (TCW // 128)):
                        t0 = tb * 128
                        g = tb % 4   # position within qscr flush group
                        if g == 0:
                            qk4 = workp.tile([128, 512], bf16, name="qk4", tag="qk4")
                        ps_qkv = psA.tile([128, 192], f32, name="ps_qkv")
                        for c in range(8):
                            nc.tensor.matmul(
                                ps_qkv, lhsT=xt_sl(c, t0, t0 + 128), rhs=w_sb[c],
                                start=(c == 0), stop=(c == 7),
                            )
                        nc.vector.tensor_tensor(
                            out=qk4[:, g * 128:g * 128 + 128], in0=ps_qkv[:, 0:128],
                            in1=bias_bc[:, 0:128], op=Alu.add)
                        nc.vector.tensor_tensor(
                            out=vext_sb[tb][:, 0:HS], in0=ps_qkv[:, 128:192],
                            in1=bias_bc[:, 128:192], op=Alu.add)
                        nc.vector.memset(vext_sb[tb][:, HS:HS + 1], 1.0)
                        # kT
                        ps_kt = psB.tile([64, 128], bf16, name="ps_kt", tag="small")
                        nc.tensor.transpose(ps_kt, qk4[:, g * 128 + 64:g * 128 + 128],
                                            ident_b)
                        nc.scalar.copy(kt_sb[:, t0:t0 + 128], ps_kt)
                        if g == 3:
                            # flush 4 t-blocks of q rows to DRAM in one SWDGE DMA
                            tq0 = (tb - 3) * 128
                            qv = qk4.rearrange("p (b z) -> p b z", b=4)[:, :, 0:HS]
                            ov = qscr[tq0:tq0 + 512, :].rearrange(
                                "(b p) d -> p b d", p=128)
                            nc.gpsimd.dma_start(out=ov, in_=qv)
                            flushed = tb + 1
                            for qc in range(NQC):
                                if qc not in wave_state and _NTB_QC[qc] <= flushed:
                                    emit_gather(qc)
                                    queue_wave(qc)
                        emit_pairs(1)
                emit_pairs(len(pair_queue))

            for _rep in range(reps):
                emit_once()

    nc.compile()
    return nc


def _get_program():
    if "nc" not in _prog_cache:
        _prog_cache["nc"] = _build_program()
    return _prog_cache["nc"]


def _host_wpack(Wq, bq, Wk, bk, Wv, bv):
    wext = np.concatenate(
        [np.asarray(Wq).T, np.asarray(Wk).T, np.asarray(Wv).T], axis=1
    ).astype(np.float32)  # [C, 192]
    wpack = np.empty((128, 8 * 192), dtype=np.float32)
    for c in range(8):
        wpack[:, c * 192:(c + 1) * 192] = wext[c * 128:(c + 1) * 128, :]
    bias = np.concatenate(
        [np.asarray(bq), np.asarray(bk), np.asarray(bv)]
    ).astype(np.float32)[None, :]  # [1, 192]
    return wpack, bias


def kernel(x, Wq, bq, Wk, bk, Wv, bv):
    from concourse.bass_utils import run_bass_kernel_spmd

    x = np.asarray(x, dtype=np.float32)
    wpack, bias = _host_wpack(Wq, bq, Wk, bk, Wv, bv)
    masks = _host_masks()
    keep_u32 = np.ascontiguousarray(
        KEEP.astype(np.uint32).reshape(_NJ, 128).T)  # [128, NJ]

    nc = _get_program()
    in_maps = []
    for b in range(NCORES):
        in_maps.append({
            "xt": np.ascontiguousarray(x[b].T).astype(BF16),
            "wpack": wpack,
            "bias": bias,
            "masks": masks,
            "keepidx": keep_u32,
        })
    res = run_bass_kernel_spmd(nc, in_maps, core_ids=list(range(NCORES)),
                               trace=TRACE, **TRACE_KW)
    global LAST_RESULTS
    LAST_RESULTS = res
    out = np.stack([res.results[b]["out"] for b in range(NCORES)], axis=0)
    return out.astype(np.float32)



# revision 7
# speedup vs baseline: 1.0781x; 1.0781x over previous
"""Sparse-attention head kernel for Trainium2, data-parallel over batch on 8 cores.

Math per batch b (see reference):
  q,k,v = x @ W{q,k,v}.T + b{q,k,v}          # [T, 64]
  qg    = q[keep]                            # [K=T/2, 64]
  att   = softmax(mask(qg @ k.T / sqrt(C)))  # [K, T], row i allows t <= keep[i]
  out   = att @ v                            # [K, 64]

Device strategy (per core, one batch):
  - host uploads x[b].T in bf16 (contraction dim C on SBUF partitions),
    one 3D DMA per 512-column t-chunk (all 8 C-chunks at once)
  - projections as qkv_nat[t,192] = sum_c xT_chunk.T @ Wchunk (+ ones x bias)
  - kT stored two-deep: even t-blocks on partitions 0-63, odd on 64-127,
    so score matmuls for a t-block pair run as two concurrent row-group
    tiles (K=64 each) on the PE array -> ~2x ST throughput
  - qgT duplicated onto both partition halves (two PE transposes into one
    PSUM tile) to feed both row groups
  - q rows for the first two q-chunks round-trip DRAM for an indirect
    gather; the last two q-chunks have contiguous keep rows (t 3072..4095)
    and transpose directly out of the projection tiles
  - masks applied additively (-1e30) on the score PSUM before exp, so the
    exp output is consumed only by the PV matmul (single-dependency chain)
  - transposed attention: S_T[t,q] in PSUM, E = exp(S/sqrt(C)) on ACT,
    out_T[65,q] = sum_t [v|1].T @ E  (row 64 = softmax denominator)
  - PE-transpose out_T, divide by denominator, DMA out
All matmul inputs bf16 (fp32 accumulation in PSUM); final epilogue in fp32.
"""

import math
import os

if "JAX_PLATFORMS" not in os.environ:
    os.environ["JAX_PLATFORMS"] = "axon,cpu"

import numpy as np
import ml_dtypes

B, T, C = 8, 4096, 1024
HS = 64
KQ = T // 2  # 2048 gathered query rows
NCORES = 8
SCALE = float(C) ** -0.5
QC = 512   # attention q-chunk (matmul moving width)
BF16 = ml_dtypes.bfloat16
NQC = KQ // QC  # 4
NEG = -1.0e30


def _keep_indices(t):
    a = math.ceil(t / 4)
    keep = [t - 1 - x for x in range(a)]
    keep += [t - 1 - math.ceil(3 / a * (x - a) ** 2 + a) for x in range(a, math.ceil(t / 2))]
    return np.array(list(reversed(keep)), dtype=np.int64)


KEEP = _keep_indices(T)  # [KQ], ascending
# last KQ/2 keep rows are exactly t = T-KQ/2 .. T-1 (contiguous)
assert (KEEP[KQ // 2:] == np.arange(T - KQ // 2, T)).all()

# Static block classification at [t=128] x [q=128] granularity.
_NT = T // 128   # 32
_NJ = KQ // 128  # 16
_FULL, _BOUND, _DEAD = 0, 1, 2
_BLOCK_KIND = np.empty((_NT, _NJ), dtype=np.int64)
_MASK_IDX = {}
for _tb in range(_NT):
    for _j in range(_NJ):
        qlo = KEEP[_j * 128]
        qhi = KEEP[_j * 128 + 127]
        if 128 * _tb + 127 <= qlo:
            _BLOCK_KIND[_tb, _j] = _FULL
        elif 128 * _tb > qhi:
            _BLOCK_KIND[_tb, _j] = _DEAD
        else:
            _BLOCK_KIND[_tb, _j] = _BOUND
            _MASK_IDX[(_tb, _j)] = len(_MASK_IDX)
_NMASK = len(_MASK_IDX)

# t-blocks needed per q-chunk (all even -> full t-block pairs)
_NTB_QC = [int(KEEP[qc * QC + QC - 1]) // 128 + 1 for qc in range(NQC)]
assert all(n % 2 == 0 for n in _NTB_QC)


def _alive_j0(qc, tb):
    # sub-blocks j in [4qc, 4qc+4); dead ones form a prefix (keep ascending)
    for jj in range(QC // 128):
        if _BLOCK_KIND[tb, qc * (QC // 128) + jj] != _DEAD:
            return jj
    return QC // 128


def _host_masks():
    # additive masks: 0 where allowed, NEG where disallowed
    m = np.zeros((128, _NMASK * 128), dtype=np.float32)
    for (tb, j), idx in _MASK_IDX.items():
        tvals = 128 * tb + np.arange(128)[:, None]
        kvals = KEEP[j * 128:(j + 1) * 128][None, :]
        m[:, idx * 128:(idx + 1) * 128] = np.where(tvals <= kvals, 0.0, NEG)
    return m.astype(BF16)


_prog_cache = {}
TRACE = False          # set by test harness to collect an NTFF profile
TRACE_KW = {}
LAST_RESULTS = None    # BassKernelResults of the most recent kernel() call


def _build_program(reps=1):
    import concourse.bass as bass
    import concourse.mybir as mybir
    import concourse.tile as tile
    from concourse import bacc
    from concourse.masks import make_identity

    dt = mybir.dt
    f32, bf16, u32 = dt.float32, dt.bfloat16, dt.uint32
    Alu = mybir.AluOpType
    Act = mybir.ActivationFunctionType

    nc = bacc.Bacc("TRN2", target_bir_lowering=False, debug=False,
                   enable_partition_id=False)

    xt_d = nc.dram_tensor("xt", [C, T], bf16, kind="ExternalInput").ap()
    wpack_d = nc.dram_tensor("wpack", [128, 8 * 192], f32, kind="ExternalInput").ap()
    bias_d = nc.dram_tensor("bias", [1, 192], f32, kind="ExternalInput").ap()
    masks_d = nc.dram_tensor("masks", [128, _NMASK * 128], bf16, kind="ExternalInput").ap()
    keep_d = nc.dram_tensor("keepidx", [128, _NJ], u32, kind="ExternalInput").ap()
    out_d = nc.dram_tensor("out", [KQ, HS], f32, kind="ExternalOutput").ap()

    NTC = 8        # xt DMA t-chunks
    TCW = T // NTC  # 512
    NPAIR = _NT // 2

    with tile.TileContext(nc) as tc:
        with (
            tc.tile_pool(name="const", bufs=1) as constp,
            tc.tile_pool(name="xt", bufs=1) as xtp,
            tc.tile_pool(name="proj", bufs=1) as projp,
            tc.tile_pool(name="dram", bufs=1, space="DRAM") as dramp,
            tc.tile_pool(name="psA", bufs=2, space="PSUM") as psA,
            tc.tile_pool(name="psB", bufs=1, space="PSUM") as psB,
            tc.tile_pool(name="psS", bufs=2, space="PSUM") as psS,
            tc.tile_pool(name="psO", bufs=1, space="PSUM") as psO,
            tc.tile_pool(name="work", bufs=2) as workp,
            tc.tile_pool(name="ework", bufs=4) as ep,
        ):
            # ---- persistent big tensors (must precede per-chunk DMAs so the
            # first x chunk can start immediately) ----
            xt_big = xtp.tile([128, 8 * T], bf16, name="xt_big", tag="xt_big")
            xt3 = xt_big.rearrange("p (c t) -> p c t", c=8)
            xt_d3 = xt_d.rearrange("(c p) t -> p c t", p=128)
            # kick off the first x chunk before anything else
            nc.sync.dma_start(out=xt3[:, :, 0:TCW], in_=xt_d3[:, :, 0:TCW])

            # ---- constants (SWDGE: keep the HWDGE queues free for xt bulk) ----
            ident_b = constp.tile([128, 128], bf16)
            make_identity(nc, ident_b)
            ident_f = constp.tile([128, 128], f32)
            make_identity(nc, ident_f)

            wpack_sb = constp.tile([128, 8 * 192], bf16)
            nc.gpsimd.dma_start(out=wpack_sb, in_=wpack_d)
            w_sb = [wpack_sb[:, c * 192:(c + 1) * 192] for c in range(8)]
            bias_bc = constp.tile([128, 192], bf16)
            nc.gpsimd.dma_start(out=bias_bc, in_=bias_d.to_broadcast([128, 192]))

            mask_big = constp.tile([128, _NMASK * 128], bf16)
            nc.gpsimd.dma_start(out=mask_big, in_=masks_d)
            keep_big = constp.tile([128, _NJ], u32)
            nc.gpsimd.dma_start(out=keep_big, in_=keep_d)

            # ---- per-repetition kernel body (reps>1 only for timing) ----
            def emit_once(first):
                # persistent tensors: same pool tags each rep -> slots reused,
                # reps serialize on the data naturally
                kt2 = projp.tile([128, NPAIR * 128], bf16, name="kt2", tag="kt2")
                qgt2 = projp.tile([128, KQ], bf16, name="qgt2", tag="qgt2")
                vext = projp.tile([128, _NT * (HS + 1)], bf16, name="vext",
                                  tag="vext")
                vext3 = vext.rearrange("p (b z) -> p b z", b=_NT)
                qscr = dramp.tile([T, HS], bf16, name="qscr", tag="qscr")

                def vext_sl(tb):
                    return vext[:, tb * (HS + 1): tb * (HS + 1) + HS + 1]

                # ones column of every [v|1] block, one strided memset
                nc.gpsimd.memset(vext3[:, :, HS:HS + 1], 1.0)

                wave_state = {}

                def emit_gather(qc):
                    # indirect row gather (q-chunks 0,1 only: scattered keep)
                    ntb = _NTB_QC[qc]
                    qsrc = qscr[0:ntb * 128, :]  # dep only on projected prefix
                    for jj in range(QC // 128):
                        j = qc * (QC // 128) + jj
                        qg_g = workp.tile([128, HS], bf16, name="qg_g", tag="qg")
                        nc.gpsimd.indirect_dma_start(
                            out=qg_g, out_offset=None, in_=qsrc,
                            in_offset=bass.IndirectOffsetOnAxis(
                                ap=keep_big[:, j:j + 1], axis=0),
                        )
                        ps_q = psA.tile([128, 128], bf16, name="ps_q",
                                        tag="ps_qkv")
                        nc.tensor.transpose(ps_q[0:64, :], qg_g, ident_b)
                        nc.tensor.transpose(ps_q[64:128, :], qg_g, ident_b,
                                            tile_position=(0, 64))
                        nc.vector.tensor_copy(qgt2[:, j * 128:(j + 1) * 128], ps_q)

                def emit_pair(qc, tba, tbb):
                    """Row-tiled ST pair + masks + one exp; emits previous
                    pair's PVs between the STs and the exp."""
                    st = wave_state[qc]
                    if st["ps_o"] is None:
                        st["ps_o"] = psO.tile([HS + 1, QC], f32, name=f"ps_o_{qc}",
                                              tag="ps_o")
                    q0 = qc * QC
                    i = tba // 2
                    a0a = _alive_j0(qc, tba) * 128
                    a0b = _alive_j0(qc, tbb) * 128
                    ps_s = psS.tile([128, 2 * QC], f32, name="ps_s")
                    nc.tensor.matmul(
                        ps_s[:, a0a:QC],
                        lhsT=kt2[0:64, i * 128:(i + 1) * 128],
                        rhs=qgt2[0:64, q0 + a0a:q0 + QC], start=True, stop=True,
                    )
                    nc.tensor.matmul(
                        ps_s[:, QC + a0b:2 * QC],
                        lhsT=kt2[64:128, i * 128:(i + 1) * 128],
                        rhs=qgt2[64:128, q0 + a0b:q0 + QC], start=True, stop=True,
                    )
                    # additive masks on PSUM (before exp) for boundary blocks
                    for i2, tb in enumerate((tba, tbb)):
                        for jj in range(QC // 128):
                            j = q0 // 128 + jj
                            if _BLOCK_KIND[tb, j] == _BOUND:
                                midx = _MASK_IDX[(tb, j)]
                                o = i2 * QC + jj * 128
                                nc.vector.tensor_tensor(
                                    out=ps_s[:, o:o + 128], in0=ps_s[:, o:o + 128],
                                    in1=mask_big[:, midx * 128:(midx + 1) * 128],
                                    op=Alu.add,
                                )
                    prev_pv = st["pv_pending"]
                    st["pv_pending"] = None
                    if prev_pv is not None:
                        emit_pv(qc, *prev_pv)
                    e_sb = ep.tile([128, 2 * QC], bf16, name="e_sb")
                    nc.scalar.activation(e_sb[:, a0a:2 * QC], ps_s[:, a0a:2 * QC],
                                         Act.Exp, scale=SCALE)
                    st["pv_pending"] = ((tba, tbb), e_sb, (a0a, a0b))

                def emit_pv(qc, tbs, e_sb, a0s):
                    st = wave_state[qc]
                    ntb = _NTB_QC[qc]
                    for i, tb in enumerate(tbs):
                        nc.tensor.matmul(
                            st["ps_o"][:, a0s[i]:QC], lhsT=vext_sl(tb),
                            rhs=e_sb[:, i * QC + a0s[i]:(i + 1) * QC],
                            start=(tb == 0), stop=(tb == ntb - 1),
                        )

                def emit_epilogue(qc):
                    st = wave_state[qc]
                    if st["pv_pending"] is not None:
                        emit_pv(qc, *st["pv_pending"])
                        st["pv_pending"] = None
                    q0 = qc * QC
                    ps_o = st["ps_o"]
                    ot_sb = workp.tile([HS + 1, QC], f32, name="ot_sb", tag="ot")
                    nc.vector.tensor_copy(ot_sb, ps_o)
                    out4 = workp.tile([128, (QC // 128) * HS], f32,
                                      name="out4", tag="out4")
                    for jj in range(QC // 128):
                        ps_on = psA.tile([128, HS + 1], f32, name="ps_on",
                                         tag="ps_qkv")
                        nc.tensor.transpose(
                            ps_on, ot_sb[:, jj * 128:(jj + 1) * 128],
                            ident_f[0:HS + 1, 0:HS + 1],
                        )
                        rec = workp.tile([128, 1], f32, name="rec", tag="rec")
                        nc.vector.reciprocal(rec, ps_on[:, HS:HS + 1])
                        nc.vector.tensor_scalar(
                            out=out4[:, jj * HS:(jj + 1) * HS], in0=ps_on[:, 0:HS],
                            scalar1=rec[:, :1], scalar2=None, op0=Alu.mult,
                        )
                    out_view = out_d[q0:q0 + QC, :].rearrange("(j p) d -> p j d", p=128)
                    nc.sync.dma_start(out=out_view,
                                      in_=out4.rearrange("p (j d) -> p j d",
                                                         j=QC // 128))

                pair_queue = []

                def emit_pairs(n):
                    for _ in range(min(n, len(pair_queue))):
                        item = pair_queue.pop(0)
                        if item[0] == "pair":
                            emit_pair(*item[1:])
                        else:
                            emit_epilogue(item[1])

                def queue_wave(qc):
                    wave_state[qc] = {"ps_o": None, "pv_pending": None}
                    ntb = _NTB_QC[qc]
                    for tb in range(0, ntb, 2):
                        pair_queue.append(("pair", qc, tb, tb + 1))
                    pair_queue.append(("epi", qc))

                # ---- load xT + projections, attention interleaved ----
                qk4 = None
                ps_kt = None
                flushed = 0
                for tci in range(NTC):
                    lo, hi = tci * TCW, (tci + 1) * TCW
                    if tci > 0 or not first:
                        nc.sync.dma_start(out=xt3[:, :, lo:hi],
                                          in_=xt_d3[:, :, lo:hi])
                    for tb in range(tci * (TCW // 128), (tci + 1) * (TCW // 128)):
                        t0 = tb * 128
                        g = tb % 4   # position within qscr flush group
                        if g == 0:
                            qk4 = workp.tile([128, 512], bf16, name="qk4", tag="qk4")
                        ps_qkv = psA.tile([128, 192], f32, name="ps_qkv")
                        for c in range(8):
                            nc.tensor.matmul(
                                ps_qkv, lhsT=xt3[:, c, t0:t0 + 128], rhs=w_sb[c],
                                start=(c == 0), stop=(c == 7),
                            )
                        nc.vector.tensor_tensor(
                            out=qk4[:, g * 128:g * 128 + 128], in0=ps_qkv[:, 0:128],
                            in1=bias_bc[:, 0:128], op=Alu.add)
                        nc.vector.tensor_tensor(
                            out=vext3[:, tb, 0:HS], in0=ps_qkv[:, 128:192],
                            in1=bias_bc[:, 128:192], op=Alu.add)
                        # kT: even t-block -> partitions 0-63, odd -> 64-127
                        half = (tb % 2) * 64
                        if half == 0:
                            ps_kt = psB.tile([128, 128], bf16, name="ps_kt",
                                             tag="small")
                        nc.tensor.transpose(
                            ps_kt[half:half + 64, :],
                            qk4[:, g * 128 + 64:g * 128 + 128], ident_b,
                            tile_position=(0, half))
                        if half == 64:
                            nc.vector.tensor_copy(
                                kt2[:, (tb // 2) * 128:(tb // 2) * 128 + 128],
                                ps_kt)
                        # contiguous keep rows (t >= 3072): qgT straight from qk4
                        if tb >= _NT - _NJ // 2:
                            j = tb - (_NT - _NJ)
                            ps_q = psA.tile([128, 128], bf16, name="ps_q",
                                            tag="ps_qkv")
                            nc.tensor.transpose(
                                ps_q[0:64, :], qk4[:, g * 128:g * 128 + HS],
                                ident_b)
                            nc.tensor.transpose(
                                ps_q[64:128, :], qk4[:, g * 128:g * 128 + HS],
                                ident_b, tile_position=(0, 64))
                            nc.vector.tensor_copy(
                                qgt2[:, j * 128:(j + 1) * 128], ps_q)
                        if g == 3 and tb < _NT - _NJ // 2:
                            # flush 4 t-blocks of q rows to DRAM in one SWDGE DMA
                            tq0 = (tb - 3) * 128
                            qv = qk4.rearrange("p (b z) -> p b z", b=4)[:, :, 0:HS]
                            ov = qscr[tq0:tq0 + 512, :].rearrange(
                                "(b p) d -> p b d", p=128)
                            nc.gpsimd.dma_start(out=ov, in_=qv)
                            flushed = tb + 1
                        # wave unlocks
                        for qc in range(2):
                            if qc not in wave_state and _NTB_QC[qc] <= flushed:
                                emit_gather(qc)
                                queue_wave(qc)
                        for qc in range(2, NQC):
                            if qc not in wave_state and _NTB_QC[qc] <= tb + 1:
                                queue_wave(qc)
                        emit_pairs(1)
                emit_pairs(len(pair_queue))

            for _rep in range(reps):
                emit_once(_rep == 0)

    nc.compile()
    return nc


def _get_program():
    if "nc" not in _prog_cache:
        _prog_cache["nc"] = _build_program()
    return _prog_cache["nc"]


def _host_wpack(Wq, bq, Wk, bk, Wv, bv):
    wext = np.concatenate(
        [np.asarray(Wq).T, np.asarray(Wk).T, np.asarray(Wv).T], axis=1
    ).astype(np.float32)  # [C, 192]
    wpack = np.empty((128, 8 * 192), dtype=np.float32)
    for c in range(8):
        wpack[:, c * 192:(c + 1) * 192] = wext[c * 128:(c + 1) * 128, :]
    bias = np.concatenate(
        [np.asarray(bq), np.asarray(bk), np.asarray(bv)]
    ).astype(np.float32)[None, :]  # [1, 192]
    return wpack, bias


def kernel(x, Wq, bq, Wk, bk, Wv, bv):
    from concourse.bass_utils import run_bass_kernel_spmd

    x = np.asarray(x, dtype=np.float32)
    wpack, bias = _host_wpack(Wq, bq, Wk, bk, Wv, bv)
    masks = _host_masks()
    keep_u32 = np.ascontiguousarray(
        KEEP.astype(np.uint32).reshape(_NJ, 128).T)  # [128, NJ]

    nc = _get_program()
    in_maps = []
    for b in range(NCORES):
        in_maps.append({
            "xt": np.ascontiguousarray(x[b].T).astype(BF16),
            "wpack": wpack,
            "bias": bias,
            "masks": masks,
            "keepidx": keep_u32,
        })
    res = run_bass_kernel_spmd(nc, in_maps, core_ids=list(range(NCORES)),
                               trace=TRACE, **TRACE_KW)
    global LAST_RESULTS
    LAST_RESULTS = res
    out = np.stack([res.results[b]["out"] for b in range(NCORES)], axis=0)
    return out.astype(np.float32)
